# revision 17
# baseline (speedup 1.0000x reference)
"""Trainium2 Bass kernel for nn_Antecedents: fuzzy-rule antecedent activations.

Computes out[n, r] = prod_v memberships[v, n, set_v(r)] over the full
Cartesian product of fuzzy sets (R = 4**6 = 4096 rules), for N = 16384
samples, data-parallel over 8 NeuronCores (2048 samples per core).

Per-core layout: sample n = p*16 + j (p = SBUF partition 0..127,
j = 0..15).  The rule index splits little-endian-last as
r = s0*1024 + s1*256 + s2*64 + s3*16 + s4*4 + s5, so the activation is
built by chained outer products from the last variable backwards:

  a16_all[:, (j,s4,s5)]  = X4 (x) X5      one stride-0-broadcast TT op
  x23[:, (j,s2,s3)]      = X2 (x) X3      one TT op
  a512[:, (jj,s2s3,s4s5)] = a16 * x23     one TT op per j-pair
  a1024[:, (s1, q)]      = a512 * X1      one TT op per j  (bf16 out)
  ot[:, s0*1024 + q]     = a1024 * X0[s0] 4 ops per j, DVE(4x)/ACT split

Output is stored bf16 (one extra rounding, max rel err ~8e-3, well
inside the 2e-2 gate) which halves the 256 MB output-write traffic;
the host gather casts back to float32.
"""

import numpy as np
from contextlib import ExitStack

import concourse.bass as bass
import concourse.tile as tile
from concourse import bacc, mybir
from concourse.bass_utils import run_bass_kernel_spmd

N_VARS = 6
N_FULL = 16384
N_SETS = 4
N_CORES = 8
N_SHARD = N_FULL // N_CORES  # 2048
P = 128
J = N_SHARD // P             # 16 samples per partition
R = N_SETS ** N_VARS         # 4096
F32 = mybir.dt.float32
BF16 = mybir.dt.bfloat16
MUL = mybir.AluOpType.mult

LAST_RESULTS = None
_CACHE = {}


def _bap(tile_ap, col_off, dims):
    """AP into a [P, W] tile starting at column col_off with explicit
    free dims [(stride_elems, count), ...] (outer -> inner; stride 0 =
    broadcast)."""
    base = tile_ap[:]
    return bass.AP(
        tensor=base.tensor,
        offset=base.offset + col_off,
        ap=[base.ap[0]] + [[s, c] for (s, c) in dims],
    )


def build_nc():
    nc = bacc.Bacc(
        "TRN2", target_bir_lowering=False, debug=False, num_devices=N_CORES
    )
    m = nc.dram_tensor(
        "memberships", [N_VARS, N_SHARD, N_SETS], F32, kind="ExternalInput"
    ).ap()
    out = nc.dram_tensor("out", [N_SHARD, R], BF16, kind="ExternalOutput").ap()
    out_v = out.rearrange("(p f) r -> p (f r)", p=P)  # [128, J*R]

    with tile.TileContext(nc) as tc, ExitStack() as ctx:
        xpool = ctx.enter_context(tc.tile_pool(name="x", bufs=1))
        spool = ctx.enter_context(tc.tile_pool(name="scratch", bufs=3))
        o1pool = ctx.enter_context(tc.tile_pool(name="o1", bufs=4))

        # X[v]: [128, 64] f32, column j*4 + s  <-  memberships[v, p*16+j, s]
        # (256 B contiguous per partition in DRAM -> one clean DMA per var)
        # Loaded v=5,4 first: the first compute op only needs X5 and X4.
        X = [None] * N_VARS
        for v in (5, 4, 3, 2, 1, 0):
            xv = xpool.tile([P, J * N_SETS], F32, tag=f"x{v}")
            nc.sync.dma_start(
                out=xv[:], in_=m[v].rearrange("(p f) s -> p (f s)", p=P)
            )
            X[v] = xv

        def sc(v, j, s):
            c = j * N_SETS + s
            return X[v][:, c : c + 1]

        # a16_all[:, j*16 + s4*4 + s5] = X4[:, j*4+s4] * X5[:, j*4+s5]
        a16_all = xpool.tile([P, J * 16], F32, tag="a16a")
        nc.vector.tensor_tensor(
            out=a16_all[:].rearrange("p (j a b) -> p j a b", j=J, a=4),
            in0=_bap(X[4], 0, [(4, J), (1, 4), (0, 4)]),
            in1=_bap(X[5], 0, [(4, J), (0, 4), (1, 4)]),
            op=MUL,
        )
        # x23[:, j*16 + s2*4 + s3] = X2[:, j*4+s2] * X3[:, j*4+s3]
        x23 = xpool.tile([P, J * 16], F32, tag="x23")
        nc.vector.tensor_tensor(
            out=x23[:].rearrange("p (j a b) -> p j a b", j=J, a=4),
            in0=_bap(X[2], 0, [(4, J), (1, 4), (0, 4)]),
            in1=_bap(X[3], 0, [(4, J), (0, 4), (1, 4)]),
            op=MUL,
        )

        # Final-expansion engine schedule: DVE FD=1024 bf16 op ~0.33 us
        # (4x mode), ACT ~1.15 us; DVE also carries the expansion chain.
        # n_dve per j chosen so both engines land at ~35 us total.
        # GpSimd measured 15 us/op here: keep it out entirely.
        def final_ops(j, a1024, ot, b, n_dve):
            for s in range(N_SETS):
                if s < n_dve:
                    nc.vector.tensor_scalar_mul(
                        ot[:, b + 1024 * s : b + 1024 * (s + 1)],
                        a1024[:],
                        sc(0, j, s),
                    )
                else:
                    nc.scalar.activation(
                        ot[:, b + 1024 * s : b + 1024 * (s + 1)],
                        a1024[:],
                        mybir.ActivationFunctionType.Copy,
                        scale=sc(0, j, s),
                    )

        def expand_pair(t, a512):
            # a1024p[:, jj*1024 + s1*256 + c] = a512[:, jj*256 + c]
            #                                   * X1[:, (2t+jj)*4 + s1]
            # One TT op per j-pair (bf16 out so the final stage runs in
            # the DVE 4x perf mode; costs one extra bf16 rounding).
            a1024p = spool.tile([P, 2048], BF16, tag="a1024")
            nc.vector.tensor_tensor(
                out=a1024p[:].rearrange(
                    "p (jj a c) -> p jj a c", jj=2, a=4
                ),
                in0=_bap(a512, 0, [(256, 2), (0, 4), (1, 256)]),
                in1=_bap(X[1], t * 8, [(4, 2), (1, 4), (0, 256)]),
                op=MUL,
            )
            return a1024p

        # a512 chunks are per j-pair; computed lazily, cached across the
        # tile plan below (a pair can span two single-j tiles).
        a512_cache = {}

        def get_a512(t):
            # a512[:, jj*256 + g*16 + k] = a16_all[:, (2t+jj)*16 + k]
            #                              * x23[:, (2t+jj)*16 + g]
            if t not in a512_cache:
                a512 = spool.tile([P, 512], F32, tag="a512")
                nc.vector.tensor_tensor(
                    out=a512[:].rearrange(
                        "p (jj g k) -> p jj g k", jj=2, g=16
                    ),
                    in0=_bap(a16_all, t * 32, [(16, 2), (0, 16), (1, 16)]),
                    in1=_bap(x23, t * 32, [(16, 2), (1, 16), (0, 16)]),
                    op=MUL,
                )
                a512_cache[t] = a512
            return a512_cache[t]

        # Tile plan: the DMA stream rate matches the compute rate
        # (~40 us each) and the per-byte DMA rate is chunk-size
        # independent, so exec time = first-ship latency + gap-free
        # stream + final drain.  All single-j tiles, each shipped in
        # two half-DMAs the moment the producing engine finishes --
        # minimal compute-to-ship latency everywhere, and the final
        # drain is ~1 us after the last op.  E-op split: 2 DVE + 2 ACT,
        # plus a third DVE op on every 4th j to balance measured costs
        # (DVE bf16 op ~0.48 us, ACT Copy ~1.37 us).
        def emit_single(j, a1024p, n_dve, n_chunks=2):
            a1024 = a1024p[:, (j % 2) * 1024 : (j % 2) * 1024 + 1024]
            ot = o1pool.tile([P, R], BF16, tag="o1")
            final_ops(j, a1024, ot, 0, n_dve)
            w = R // n_chunks
            for c in range(n_chunks):
                nc.sync.dma_start(
                    out=out_v[:, j * R + c * w : j * R + (c + 1) * w],
                    in_=ot[:, c * w : (c + 1) * w],
                )

        for t in range(J // 2):
            a1024p = expand_pair(t, get_a512(t))
            for jj in range(2):
                j = 2 * t + jj
                emit_single(j, a1024p, 3 if j == 15 else 2)

    nc.compile()
    return nc


def _get_nc():
    if "nc" not in _CACHE:
        _CACHE["nc"] = build_nc()
    return _CACHE["nc"]


def kernel(memberships):
    global LAST_RESULTS
    m = np.ascontiguousarray(np.asarray(memberships, dtype=np.float32))
    assert m.shape == (N_VARS, N_FULL, N_SETS), m.shape
    nc = _get_nc()
    shards = np.split(m, N_CORES, axis=1)
    in_maps = [{"memberships": np.ascontiguousarray(s)} for s in shards]
    res = run_bass_kernel_spmd(nc, in_maps, core_ids=list(range(N_CORES)))
    LAST_RESULTS = res
    return np.concatenate(
        [res.results[i]["out"] for i in range(N_CORES)], axis=0
    ).astype(np.float32)


# revision 18
# speedup vs baseline: 1.1153x; 1.1153x over previous
"""Trainium2 Bass kernel for nn_Antecedents: fuzzy-rule antecedent activations.

Computes out[n, r] = prod_v memberships[v, n, set_v(r)] over the full
Cartesian product of fuzzy sets (R = 4**6 = 4096 rules), for N = 16384
samples, data-parallel over 8 NeuronCores (2048 samples per core).

Per-core layout: sample n = p*16 + j (p = SBUF partition 0..127,
j = 0..15).  The rule index splits little-endian-last as
r = s0*1024 + s1*256 + s2*64 + s3*16 + s4*4 + s5, so the activation is
built by chained outer products from the last variable backwards:

  a16_all[:, (j,s4,s5)]  = X4 (x) X5      one stride-0-broadcast TT op
  x23[:, (j,s2,s3)]      = X2 (x) X3      one TT op
  a512[:, (jj,s2s3,s4s5)] = a16 * x23     one TT op per j-pair
  a1024[:, (s1, q)]      = a512 * X1      one TT op per j  (bf16 out)
  ot[:, s0*1024 + q]     = a1024 * X0[s0] 4 ops per j, DVE(4x)/ACT split

Output is stored bf16 (one extra rounding, max rel err ~8e-3, well
inside the 2e-2 gate) which halves the 256 MB output-write traffic;
the host gather casts back to float32.
"""

import numpy as np
from contextlib import ExitStack

import concourse.bass as bass
import concourse.tile as tile
from concourse import bacc, mybir
from concourse.bass_utils import run_bass_kernel_spmd

N_VARS = 6
N_FULL = 16384
N_SETS = 4
N_CORES = 8
N_SHARD = N_FULL // N_CORES  # 2048
P = 128
J = N_SHARD // P             # 16 samples per partition
R = N_SETS ** N_VARS         # 4096
F32 = mybir.dt.float32
BF16 = mybir.dt.bfloat16
MUL = mybir.AluOpType.mult

LAST_RESULTS = None
_CACHE = {}


def _bap(tile_ap, col_off, dims):
    """AP into a [P, W] tile starting at column col_off with explicit
    free dims [(stride_elems, count), ...] (outer -> inner; stride 0 =
    broadcast)."""
    base = tile_ap[:]
    return bass.AP(
        tensor=base.tensor,
        offset=base.offset + col_off,
        ap=[base.ap[0]] + [[s, c] for (s, c) in dims],
    )


def build_nc():
    nc = bacc.Bacc(
        "TRN2", target_bir_lowering=False, debug=False, num_devices=N_CORES
    )
    m = nc.dram_tensor(
        "memberships", [N_VARS, N_SHARD, N_SETS], F32, kind="ExternalInput"
    ).ap()
    out = nc.dram_tensor("out", [N_SHARD, R], BF16, kind="ExternalOutput").ap()
    out_v = out.rearrange("(p f) r -> p (f r)", p=P)  # [128, J*R]

    with tile.TileContext(nc) as tc, ExitStack() as ctx:
        xpool = ctx.enter_context(tc.tile_pool(name="x", bufs=1))
        spool = ctx.enter_context(tc.tile_pool(name="scratch", bufs=3))
        o1pool = ctx.enter_context(tc.tile_pool(name="o1", bufs=4))

        # X[v]: [128, 64] f32, column j*4 + s  <-  memberships[v, p*16+j, s]
        # (256 B contiguous per partition in DRAM -> one clean DMA per var)
        # Loaded v=5,4 first: the first compute op only needs X5 and X4.
        X = [None] * N_VARS
        for v in (5, 4, 3, 2, 1, 0):
            xv = xpool.tile([P, J * N_SETS], F32, tag=f"x{v}")
            nc.sync.dma_start(
                out=xv[:], in_=m[v].rearrange("(p f) s -> p (f s)", p=P)
            )
            X[v] = xv

        def sc(v, j, s):
            c = j * N_SETS + s
            return X[v][:, c : c + 1]

        # a16_all[:, j*16 + s4*4 + s5] = X4[:, j*4+s4] * X5[:, j*4+s5]
        a16_all = xpool.tile([P, J * 16], F32, tag="a16a")
        nc.vector.tensor_tensor(
            out=a16_all[:].rearrange("p (j a b) -> p j a b", j=J, a=4),
            in0=_bap(X[4], 0, [(4, J), (1, 4), (0, 4)]),
            in1=_bap(X[5], 0, [(4, J), (0, 4), (1, 4)]),
            op=MUL,
        )
        # x23[:, j*16 + s2*4 + s3] = X2[:, j*4+s2] * X3[:, j*4+s3]
        x23 = xpool.tile([P, J * 16], F32, tag="x23")
        nc.vector.tensor_tensor(
            out=x23[:].rearrange("p (j a b) -> p j a b", j=J, a=4),
            in0=_bap(X[2], 0, [(4, J), (1, 4), (0, 4)]),
            in1=_bap(X[3], 0, [(4, J), (0, 4), (1, 4)]),
            op=MUL,
        )

        # Final-expansion engine schedule: DVE FD=1024 bf16 op ~0.33 us
        # (4x mode), ACT ~1.15 us; DVE also carries the expansion chain.
        # n_dve per j chosen so both engines land at ~35 us total.
        # GpSimd measured 15 us/op here: keep it out entirely.
        def final_ops(j, a1024, ot, b, n_dve):
            for s in range(N_SETS):
                if s < n_dve:
                    nc.vector.tensor_scalar_mul(
                        ot[:, b + 1024 * s : b + 1024 * (s + 1)],
                        a1024[:],
                        sc(0, j, s),
                    )
                else:
                    nc.scalar.activation(
                        ot[:, b + 1024 * s : b + 1024 * (s + 1)],
                        a1024[:],
                        mybir.ActivationFunctionType.Copy,
                        scale=sc(0, j, s),
                    )

        def expand_j(j, jj, a512):
            # a1024[:, s1*256 + c] = a512[:, jj*256 + c] * X1[:, j*4+s1]
            # (bf16 out so the final stage runs in the DVE 4x perf mode;
            # costs one extra bf16 rounding on top of the output one)
            a1024 = spool.tile([P, 1024], BF16, tag="a1024")
            nc.vector.tensor_tensor(
                out=a1024[:].rearrange("p (a c) -> p a c", a=4),
                in0=_bap(a512, jj * 256, [(0, 4), (1, 256)]),
                in1=_bap(X[1], j * 4, [(1, 4), (0, 256)]),
                op=MUL,
            )
            return a1024

        # a512 chunks are per j-pair; computed lazily, cached across the
        # tile plan below (a pair can span two single-j tiles).
        a512_cache = {}

        def get_a512(t):
            # a512[:, jj*256 + g*16 + k] = a16_all[:, (2t+jj)*16 + k]
            #                              * x23[:, (2t+jj)*16 + g]
            if t not in a512_cache:
                a512 = spool.tile([P, 512], F32, tag="a512")
                nc.vector.tensor_tensor(
                    out=a512[:].rearrange(
                        "p (jj g k) -> p jj g k", jj=2, g=16
                    ),
                    in0=_bap(a16_all, t * 32, [(16, 2), (0, 16), (1, 16)]),
                    in1=_bap(x23, t * 32, [(16, 2), (1, 16), (0, 16)]),
                    op=MUL,
                )
                a512_cache[t] = a512
            return a512_cache[t]

        # Tile plan: the DMA stream rate matches the compute rate
        # (~40 us each) and the per-byte DMA rate is chunk-size
        # independent, so exec time = first-ship latency + gap-free
        # stream + final drain.  All single-j tiles, each shipped in
        # two half-DMAs the moment the producing engine finishes --
        # minimal compute-to-ship latency everywhere, and the final
        # drain is ~1 us after the last op.  E-op split: 2 DVE + 2 ACT,
        # plus a third DVE op on every 4th j to balance measured costs
        # (DVE bf16 op ~0.48 us, ACT Copy ~1.37 us).
        def emit_single(j, n_dve, n_chunks=2):
            a1024 = expand_j(j, j % 2, get_a512(j // 2))
            ot = o1pool.tile([P, R], BF16, tag="o1")
            final_ops(j, a1024, ot, 0, n_dve)
            w = R // n_chunks
            for c in range(n_chunks):
                nc.sync.dma_start(
                    out=out_v[:, j * R + c * w : j * R + (c + 1) * w],
                    in_=ot[:, c * w : (c + 1) * w],
                )

        for j in range(J):
            emit_single(j, 3 if j % 4 == 3 else 2)

    nc.compile()
    return nc


def _get_nc():
    if "nc" not in _CACHE:
        _CACHE["nc"] = build_nc()
    return _CACHE["nc"]


def kernel(memberships):
    global LAST_RESULTS
    m = np.ascontiguousarray(np.asarray(memberships, dtype=np.float32))
    assert m.shape == (N_VARS, N_FULL, N_SETS), m.shape
    nc = _get_nc()
    shards = np.split(m, N_CORES, axis=1)
    in_maps = [{"memberships": np.ascontiguousarray(s)} for s in shards]
    res = run_bass_kernel_spmd(nc, in_maps, core_ids=list(range(N_CORES)))
    LAST_RESULTS = res
    return np.concatenate(
        [res.results[i]["out"] for i in range(N_CORES)], axis=0
    ).astype(np.float32)
